# revision 10
# baseline (speedup 1.0000x reference)
"""ACMConv (adaptive channel mixing GCN layer) on 8 Trainium2 NeuronCores.

Strategy (graph/data parallel, edges partitioned by destination):
- Host: compute GCN norms with self loops; factorize
  norm_e = dis[src] * dis[dst]. The src factor is folded into the gather
  table xd = dis[:,None]*x (bf16); the dst factor is applied in phase 2.
  Non-loop edges are partitioned by destination core (node blocks of
  12500), grouped into 128-destination windows, and split by source bank
  (int16 gather indices address 32768-row banks). Each (window, bank)
  segment is padded to M_b 128-edge tiles, uniform across cores (SPMD).
- Device phase 1: per (14-window block, bank) one dma_gather pulls all
  source rows (idx 0 for padding); one wide tensor_tensor builds the
  one-hot eq[e, j] = (dest_off_e == j) (999 offsets kill padding); the
  tensor engine accumulates psum[f, j] += xg.T @ eq over the window's
  4*M_b tiles, yielding the window's aggregation s~ pre-transposed; an
  ACT copy casts it to bf16 staging sT~[128, nodes].
- Device phase 2: per 128-node tile, s~ += dis*x (self loops, host-shipped
  xdT), then bf16 matmuls s~@W_low.T, s~@W_high.T, x@W_high.T, x@W_id.T,
  x@W_gate.T (+ bias rank-1 terms via K=1 matmuls, z~-weighted for
  aggregated terms), softmax gate, and combine with the dst factor dis
  folded into the gate coefficients of the aggregated terms.
Output rows are node-contiguous so the host just concatenates core shards.
"""

import numpy as np
import ml_dtypes

import concourse.bass as bass
import concourse.bacc as bacc
import concourse.mybir as mybir
import concourse.tile as tile
from concourse.bass_utils import run_bass_kernel_spmd

N_NODES = 100000
D = 128
NCORES = 8
NPC = N_NODES // NCORES            # 12500 nodes per core
P = 128
NWIN = (NPC + P - 1) // P          # 98 destination windows per core
NPC_PAD = NWIN * P                 # 12544
BW = 14                            # windows per block
NB = NWIN // BW                    # 7 blocks
NBANK = 4
BANK = 32768
PAD_OFF = 999.0                    # kills one-hot for padding slots
assert NB * BW == NWIN

F32 = mybir.dt.float32
BF16 = mybir.dt.bfloat16
I16 = mybir.dt.int16
NPBF = ml_dtypes.bfloat16


# ---------------------------------------------------------------- host side


def _graph_prep(edge_index):
    ei = np.asarray(edge_index).astype(np.int64)
    row, col = ei[0], ei[1]
    loops = np.arange(N_NODES, dtype=np.int64)
    deg = np.bincount(np.concatenate([row, loops]), minlength=N_NODES)
    dis = (1.0 / np.sqrt(np.maximum(deg, 1.0))).astype(np.float64)
    # z~_j = sum_{in-edges} dis[src] + dis_j (self); z_j = dis_j * z~_j
    zt_all = np.bincount(col, weights=dis[row], minlength=N_NODES) + dis
    zt_all = zt_all.astype(np.float32)
    dis = dis.astype(np.float32)

    core = col // NPC
    per_core = []
    maxcnt = 0
    for c in range(NCORES):
        msk = core == c
        r_c = row[msk]
        d_c = col[msk] - c * NPC
        win = d_c // P
        bank = r_c // BANK
        key = (win * NBANK + bank).astype(np.int64)
        order = np.argsort(key, kind="stable")
        r_c, d_c, key = r_c[order], d_c[order], key[order]
        cnt = np.bincount(key, minlength=NWIN * NBANK)
        maxcnt = max(maxcnt, int(cnt.max()))
        per_core.append((r_c, d_c, key, cnt))
    M_b = (maxcnt + P - 1) // P
    CAP = M_b * P                   # slots per (window, bank)
    G = max(1, 1024 // (NBANK * 0 + CAP) if CAP <= 512 else 1)
    G = 2 if 2 * CAP <= 1024 else 1  # windows per gather call
    assert BW % G == 0
    NWP = BW // G                   # gather calls per (block, bank)
    CW16 = G * CAP // 16            # idx columns per call
    TPB = BW * M_b                  # tiles per (block, bank)

    eis, efs = [], []
    for c in range(NCORES):
        r_c, d_c, key, cnt = per_core[c]
        starts = np.zeros(NWIN * NBANK, np.int64)
        starts[1:] = np.cumsum(cnt)[:-1]
        slot = np.arange(len(key)) - starts[key]
        win = key // NBANK
        bank = key % NBANK
        # global slot id: (win, bank) -> slots
        idx16 = np.zeros((NWIN, NBANK, CAP), np.int16)
        offv = np.full((NWIN, NBANK, CAP), PAD_OFF, np.float32)
        idx16[win, bank, slot] = (r_c - bank * BANK).astype(np.int16)
        offv[win, bank, slot] = (d_c % P).astype(np.float32)
        # per (block, bank, window-pair) gather call of G*CAP idxs (<=1024)
        # idx layout: [NB, 128, NBANK*NWP*CW16]; idx j of call -> [j%16 (+16q), .. + j//16]
        A = idx16.reshape(NB, NWP, G, NBANK, CAP).transpose(0, 3, 1, 2, 4)
        A = A.reshape(NB, NBANK, NWP, G * CAP)
        wrapped = np.zeros((NB, NBANK, NWP, 16, G * CAP // 16), np.int16)
        j = np.arange(G * CAP)
        wrapped[:, :, :, j % 16, j // 16] = A[:, :, :, j]
        wrapped = np.tile(wrapped, (1, 1, 1, 8, 1))  # replicate for 8 Q7 cores
        ei_l = np.ascontiguousarray(
            wrapped.transpose(0, 3, 1, 2, 4).reshape(NB, 128, NBANK * NWP * CW16)
        )
        # off layout: [NB, 128, NBANK*TPB]; col = b*TPB + w'*M_b + m
        O = offv.reshape(NB, BW, NBANK, M_b, P).transpose(0, 4, 2, 1, 3)
        ef_l = np.ascontiguousarray(
            O.reshape(NB, P, NBANK * TPB).astype(NPBF)
        )
        eis.append(ei_l)
        efs.append(ef_l)
    return M_b, eis, efs, zt_all, dis


# -------------------------------------------------------------- device graph

_GRAPH_CACHE = {}


def _build(M_b):
    if M_b in _GRAPH_CACHE:
        return _GRAPH_CACHE[M_b]
    CAP = M_b * P
    G = 2 if 2 * CAP <= 1024 else 1
    NWP = BW // G
    CW16 = G * CAP // 16
    TPB = BW * M_b
    WIDE = TPB * P

    nc = bacc.Bacc(num_swdge_queues=NBANK)
    xd_ext = nc.declare_dram_parameter("xd", [N_NODES, D], BF16, isOutput=False)
    xT_ext = nc.declare_dram_parameter("xT", [D, NPC_PAD], BF16, isOutput=False)
    xdT_ext = nc.declare_dram_parameter("xdT", [D, NPC_PAD], BF16, isOutput=False)
    ei_ext = nc.declare_dram_parameter("ei", [NB, P, NBANK * NWP * CW16], I16, isOutput=False)
    ef_ext = nc.declare_dram_parameter("ef", [NB, P, NBANK * TPB], BF16, isOutput=False)
    z_ext = nc.declare_dram_parameter("z", [NB, 1, BW * P], BF16, isOutput=False)
    dis_ext = nc.declare_dram_parameter("disc", [P, NWIN], F32, isOutput=False)
    w_ext = nc.declare_dram_parameter("wmat", [P, 3 * D + 3], BF16, isOutput=False)
    b_ext = nc.declare_dram_parameter("bvec", [1, 3 * D + 3], BF16, isOutput=False)
    c_ext = nc.declare_dram_parameter("iotaw", [P, WIDE], BF16, isOutput=False)
    out_ext = nc.declare_dram_parameter("out", [NPC_PAD, D], F32, isOutput=True)
    stagT = nc.dram_tensor("stagT", [D, NPC_PAD], BF16)

    AL = mybir.AluOpType
    with tile.TileContext(nc) as tc:
        with (
            tc.tile_pool(name="const", bufs=1) as constp,
            tc.tile_pool(name="eib", bufs=2) as eip,
            tc.tile_pool(name="efb", bufs=2) as efp,
            tc.tile_pool(name="xg", bufs=2) as xgp,
            tc.tile_pool(name="eq", bufs=1) as eqp,
            tc.tile_pool(name="s1", bufs=4) as s1p,
            tc.tile_pool(name="zb", bufs=2) as zp,
            tc.tile_pool(name="p2in", bufs=3) as p2inp,
            tc.tile_pool(name="gate", bufs=3) as gatep,
            tc.tile_pool(name="comb", bufs=3) as combp,
            tc.tile_pool(name="ps_acc", bufs=3, space="PSUM") as pp_acc,
            tc.tile_pool(name="ps_mm", bufs=1, space="PSUM") as pp_mm,
        ):
            iota_w = constp.tile([P, WIDE], BF16)
            nc.sync.dma_start(out=iota_w[:], in_=c_ext[:])
            wm = constp.tile([P, 3 * D + 3], BF16)
            nc.sync.dma_start(out=wm[:], in_=w_ext[:])
            WlT = wm[:, 0:D]
            WhT = wm[:, D : 2 * D]
            WiT = wm[:, 2 * D : 3 * D]
            WgT = wm[:, 3 * D : 3 * D + 3]
            bv = constp.tile([1, 3 * D + 3], BF16)
            nc.sync.dma_start(out=bv[:], in_=b_ext[:])
            b_low = bv[:, 0:D]
            b_high = bv[:, D : 2 * D]
            b_id = bv[:, 2 * D : 3 * D]
            b_gate = bv[:, 3 * D : 3 * D + 3]
            ones = constp.tile([1, P], BF16)
            nc.vector.memset(ones[:], 1.0)
            dis_sb = constp.tile([P, NWIN], F32)
            nc.sync.dma_start(out=dis_sb[:], in_=dis_ext[:])

            # ---- phase 1: windowed segment sum of dis_src * x[src]
            for nb in range(NB):
                ei_sb = eip.tile([P, NBANK * NWP * CW16], I16)
                nc.sync.dma_start(out=ei_sb[:], in_=ei_ext[nb])
                ef_sb = efp.tile([P, NBANK * TPB], BF16)
                nc.sync.dma_start(out=ef_sb[:], in_=ef_ext[nb])
                eq = []
                for b in range(NBANK):
                    eqt = eqp.tile([P, TPB, P], BF16, tag=f"eq{b}")
                    nc.vector.tensor_tensor(
                        out=eqt[:],
                        in0=iota_w[:].rearrange("p (t j) -> p t j", j=P),
                        in1=ef_sb[:, b * TPB : (b + 1) * TPB].to_broadcast(
                            [P, TPB, P]
                        ),
                        op=AL.is_equal,
                    )
                    eq.append(eqt)
                for wp in range(NWP):
                    xg = []
                    for b in range(NBANK):
                        xgt = xgp.tile([P, G * M_b, P], BF16, tag=f"xg{b}")
                        nc.gpsimd.dma_gather(
                            out_ap=xgt[:],
                            in_ap=xd_ext[b * BANK : min((b + 1) * BANK, N_NODES), :],
                            idxs_ap=ei_sb[:, (b * NWP + wp) * CW16 : (b * NWP + wp + 1) * CW16],
                            num_idxs=G * CAP,
                            num_idxs_reg=G * CAP,
                            elem_size=P,
                            queue_num=b,
                        )
                        xg.append(xgt)
                    for g_ in range(G):
                        w_ = wp * G + g_
                        w = nb * BW + w_
                        ps = pp_acc.tile([P, P], F32, tag="ps")   # [f, j]
                        k = 0
                        for b in range(NBANK):
                            for m in range(M_b):
                                nc.tensor.matmul(
                                    ps[:],
                                    lhsT=xg[b][:, g_ * M_b + m, :],
                                    rhs=eq[b][:, w_ * M_b + m, :],
                                    start=(k == 0),
                                    stop=(k == NBANK * M_b - 1),
                                )
                                k += 1
                        s_sb = s1p.tile([P, P], BF16, tag="s1")
                        nc.scalar.copy(s_sb[:], ps[:])
                        nc.sync.dma_start(
                            out=stagT[:, w * P : (w + 1) * P], in_=s_sb[:]
                        )

            # ---- phase 2: projections, gate, combine
            for nb in range(NB):
                z_sb = zp.tile([1, BW * P], BF16, tag="z")
                nc.scalar.dma_start(out=z_sb[:], in_=z_ext[nb])
                for w_ in range(BW):
                    t = nb * BW + w_
                    c0 = t * P
                    xT_sb = p2inp.tile([P, P], BF16, tag="xT")
                    nc.scalar.dma_start(out=xT_sb[:], in_=xT_ext[:, c0 : c0 + P])
                    st_sb = p2inp.tile([P, P], BF16, tag="st")
                    nc.scalar.dma_start(out=st_sb[:], in_=stagT[:, c0 : c0 + P])
                    xdT_sb = p2inp.tile([P, P], BF16, tag="xdT")
                    nc.scalar.dma_start(out=xdT_sb[:], in_=xdT_ext[:, c0 : c0 + P])
                    sT2 = p2inp.tile([P, P], BF16, tag="sT2")
                    nc.vector.tensor_tensor(
                        out=sT2[:], in0=st_sb[:], in1=xdT_sb[:], op=AL.add
                    )
                    zrow = z_sb[0:1, w_ * P : (w_ + 1) * P]

                    ps_low = pp_mm.tile([P, P], F32, tag="ps_low")
                    nc.tensor.matmul(ps_low[:], lhsT=sT2[:], rhs=WlT, start=True, stop=False)
                    nc.tensor.matmul(ps_low[:], lhsT=zrow, rhs=b_low, start=False, stop=True)
                    ps_hl = pp_mm.tile([P, P], F32, tag="ps_hl")
                    nc.tensor.matmul(ps_hl[:], lhsT=sT2[:], rhs=WhT, start=True, stop=False)
                    nc.tensor.matmul(ps_hl[:], lhsT=zrow, rhs=b_high, start=False, stop=True)
                    ps_high = pp_mm.tile([P, P], F32, tag="ps_high")
                    nc.tensor.matmul(ps_high[:], lhsT=xT_sb[:], rhs=WhT, start=True, stop=False)
                    nc.tensor.matmul(ps_high[:], lhsT=ones[:], rhs=b_high, start=False, stop=True)
                    ps_id = pp_mm.tile([P, P], F32, tag="ps_id")
                    nc.tensor.matmul(ps_id[:], lhsT=xT_sb[:], rhs=WiT, start=True, stop=False)
                    nc.tensor.matmul(ps_id[:], lhsT=ones[:], rhs=b_id, start=False, stop=True)
                    ps_gate = pp_mm.tile([P, 3], F32, tag="ps_gate")
                    nc.tensor.matmul(ps_gate[:], lhsT=xT_sb[:], rhs=WgT, start=True, stop=False)
                    nc.tensor.matmul(ps_gate[:], lhsT=ones[:], rhs=b_gate, start=False, stop=True)

                    eg = gatep.tile([P, 3], F32, tag="eg")
                    nc.scalar.activation(
                        eg[:], ps_gate[:], mybir.ActivationFunctionType.Exp
                    )
                    gs = gatep.tile([P, 1], F32, tag="gs")
                    nc.vector.tensor_reduce(
                        out=gs[:], in_=eg[:], axis=mybir.AxisListType.X, op=AL.add
                    )
                    gr = gatep.tile([P, 1], F32, tag="gr")
                    nc.vector.reciprocal(gr[:], gs[:])
                    g = gatep.tile([P, 3], F32, tag="g")
                    nc.vector.tensor_scalar(
                        out=g[:], in0=eg[:], scalar1=gr[:, 0:1], scalar2=None,
                        op0=AL.mult,
                    )
                    # dst-side dis folds into the gate coefs of aggregated terms
                    gdis = gatep.tile([P, 2], F32, tag="gdis")
                    nc.vector.tensor_scalar(
                        out=gdis[:], in0=g[:, 0:2], scalar1=dis_sb[:, t : t + 1],
                        scalar2=None, op0=AL.mult,
                    )

                    u = combp.tile([P, P], F32, tag="u")
                    nc.scalar.activation(
                        u[:], ps_low[:], mybir.ActivationFunctionType.Copy,
                        scale=gdis[:, 0:1],
                    )
                    v1 = combp.tile([P, P], F32, tag="v1")
                    nc.scalar.activation(
                        v1[:], ps_high[:], mybir.ActivationFunctionType.Copy,
                        scale=g[:, 1:2],
                    )
                    v2 = combp.tile([P, P], F32, tag="v2")
                    nc.vector.tensor_scalar(
                        out=v2[:], in0=ps_hl[:], scalar1=gdis[:, 1:2], scalar2=None,
                        op0=AL.mult,
                    )
                    w2 = combp.tile([P, P], F32, tag="w2")
                    nc.scalar.activation(
                        w2[:], ps_id[:], mybir.ActivationFunctionType.Copy,
                        scale=g[:, 2:3],
                    )
                    o = combp.tile([P, P], F32, tag="o")
                    nc.vector.tensor_tensor(out=o[:], in0=u[:], in1=v1[:], op=AL.add)
                    nc.vector.tensor_tensor(out=o[:], in0=o[:], in1=v2[:], op=AL.subtract)
                    nc.vector.tensor_tensor(out=o[:], in0=o[:], in1=w2[:], op=AL.add)
                    nc.sync.dma_start(out=out_ext[c0 : c0 + P, :], in_=o[:])

    nc.compile()
    _GRAPH_CACHE[M_b] = nc
    return nc


# -------------------------------------------------------------------- entry


def kernel(x, edge_index, W_low, b_low, W_high, b_high, W_id, b_id, W_gate, b_gate):
    x = np.asarray(x, dtype=np.float32)
    M_b, eis, efs, zt_all, dis = _graph_prep(edge_index)
    nc = _build(M_b)
    TPB = BW * M_b

    xd = np.ascontiguousarray((dis[:, None] * x).astype(NPBF))
    wmat = np.ascontiguousarray(
        np.concatenate(
            [
                np.asarray(W_low, np.float32).T,
                np.asarray(W_high, np.float32).T,
                np.asarray(W_id, np.float32).T,
                np.asarray(W_gate, np.float32).T,
            ],
            axis=1,
        ).astype(NPBF)
    )
    bvec = np.ascontiguousarray(
        np.concatenate(
            [
                np.asarray(b_low, np.float32),
                np.asarray(b_high, np.float32),
                np.asarray(b_id, np.float32),
                np.asarray(b_gate, np.float32),
            ]
        )[None, :].astype(NPBF)
    )
    iotaw = np.ascontiguousarray(
        np.tile(np.arange(P, dtype=np.float32), (P, TPB)).astype(NPBF)
    )

    in_maps = []
    for c in range(NCORES):
        lo = c * NPC
        xp = np.zeros((NPC_PAD, D), np.float32)
        xp[:NPC] = x[lo : lo + NPC]
        xT = np.ascontiguousarray(xp.T.astype(NPBF))
        xdp = np.zeros((NPC_PAD, D), np.float32)
        xdp[:NPC] = dis[lo : lo + NPC, None] * x[lo : lo + NPC]
        xdT = np.ascontiguousarray(xdp.T.astype(NPBF))
        zt = np.zeros(NPC_PAD, np.float32)
        zt[:NPC] = zt_all[lo : lo + NPC]
        zt = np.ascontiguousarray(zt.reshape(NB, 1, BW * P).astype(NPBF))
        dc = np.zeros(NPC_PAD, np.float32)
        dc[:NPC] = dis[lo : lo + NPC]
        disc = np.ascontiguousarray(dc.reshape(NWIN, P).T)
        in_maps.append(
            dict(
                xd=xd,
                xT=xT,
                xdT=xdT,
                ei=eis[c],
                ef=efs[c],
                z=zt,
                disc=disc,
                wmat=wmat,
                bvec=bvec,
                iotaw=iotaw,
            )
        )

    try:
        res = run_bass_kernel_spmd(nc, in_maps, list(range(NCORES)))
    except Exception:
        # a previous crashed run may have wedged the device; reset and retry
        try:
            import ctypes
            import jax

            lib = ctypes.CDLL("/opt/axon/libaxon_pjrt.so")
            if hasattr(lib, "axon_reset"):
                jax.devices()
                lib.axon_reset.restype = ctypes.c_int64
                lib.axon_reset()
        except Exception:
            pass
        res = run_bass_kernel_spmd(nc, in_maps, list(range(NCORES)))
    out = np.concatenate(
        [res.results[c]["out"][:NPC] for c in range(NCORES)], axis=0
    )
    return out


# revision 12
# speedup vs baseline: 2.5085x; 2.5085x over previous
"""ACMConv (adaptive channel mixing GCN layer) on 8 Trainium2 NeuronCores.

Strategy (graph/data parallel, edges partitioned by destination):
- Host: compute GCN norms with self loops; factorize
  norm_e = dis[src] * dis[dst]. The src factor is folded into the bf16
  gather table xd = dis[:,None]*x; the dst factor is applied in phase 2.
  Non-loop edges are partitioned by destination core (node blocks of
  12500), grouped into 128-destination windows, and split by source bank
  (int16 dma_gather indices address 32768-row banks). Each (window, bank)
  segment is padded to M_wb 128-edge tiles, where M_wb is the max over
  the 8 cores for that (window, bank) -> identical SPMD graph, minimal
  padding. Segments of consecutive windows are packed into dma_gather
  calls of <= 1024 indices, issued on SWDGE queue b (4 Q7 pairs work in
  parallel).
- Device phase 1: per call one dma_gather pulls the source rows (idx 0
  for padding); per (block, bank) one wide tensor_tensor builds the
  one-hot eq[e, j] = (dest_off_e == j) (999 offsets kill padding); the
  tensor engine accumulates psum[f, j] += xg.T @ eq over the window's
  tiles, yielding the aggregation s~ pre-transposed; an ACT copy casts it
  to bf16 staging sT~[128, nodes].
- Device phase 2: per 128-node tile, s~ += dis*x (self loops, host-shipped
  xdT), then bf16 matmuls s~@W_low.T, s~@W_high.T, x@W_high.T, x@W_id.T,
  x@W_gate.T (+ bias rank-1 terms via K=1 matmuls, z~-weighted for
  aggregated terms), softmax gate, and combine with the dst factor dis
  folded into the gate coefficients of the aggregated terms.
Output rows are node-contiguous so the host just concatenates core shards.
"""

import numpy as np
import ml_dtypes

import concourse.bass as bass
import concourse.bacc as bacc
import concourse.mybir as mybir
import concourse.tile as tile
from concourse.bass_utils import run_bass_kernel_spmd

N_NODES = 100000
D = 128
NCORES = 8
NPC = N_NODES // NCORES            # 12500 nodes per core
P = 128
NWIN = (NPC + P - 1) // P          # 98 destination windows per core
NPC_PAD = NWIN * P                 # 12544
BW = 14                            # windows per block
NB = NWIN // BW                    # 7 blocks
NBANK = 4
BANK = 25000                      # equal banks -> balanced (window, bank) cells
MAX_CALL = 1024                    # dma_gather Q7 scratch limit
PAD_OFF = 999.0                    # kills one-hot for padding slots
assert NB * BW == NWIN

F32 = mybir.dt.float32
BF16 = mybir.dt.bfloat16
I16 = mybir.dt.int16
NPBF = ml_dtypes.bfloat16


class Layout:
    """Derived, deterministic layout from the (window, bank) tile map."""

    def __init__(self, M_wb):
        self.M_wb = M_wb                       # [NWIN][NBANK] ints
        # ef tile columns: per (block, bank) ragged run of tiles
        self.ef_base = {}                      # (nb, b, w_) -> tile col base
        self.tpb = {}                          # (nb, b) -> tiles in block/bank
        self.ef_blk_base = {}                  # (nb, b) -> global ef col base
        col = 0
        self.max_tpb = 0
        for nb in range(NB):
            for b in range(NBANK):
                self.ef_blk_base[(nb, b)] = col
                t = 0
                for w_ in range(BW):
                    self.ef_base[(nb, b, w_)] = t
                    t += M_wb[nb * BW + w_][b]
                self.tpb[(nb, b)] = t
                self.max_tpb = max(self.max_tpb, t)
                col += t
        self.ef_tot = col
        # gather calls: per (block, bank) greedy pack windows, <=1024 idxs
        self.calls = {}                        # (nb, b) -> list of (w0, nwin, nidx)
        self.ei_base = {}                      # (nb, b, call_i) -> i16 col base
        icol = 0
        for nb in range(NB):
            for b in range(NBANK):
                cl = []
                w_ = 0
                while w_ < BW:
                    n = 0
                    nidx = 0
                    while w_ + n < BW:
                        cap = M_wb[nb * BW + w_ + n][b] * P
                        if n > 0 and nidx + cap > MAX_CALL:
                            break
                        assert cap <= MAX_CALL
                        nidx += cap
                        n += 1
                    if nidx > 0:
                        self.ei_base[(nb, b, len(cl))] = icol
                        icol += nidx // 16
                        cl.append((w_, n, nidx))
                    w_ += max(n, 1)
                self.calls[(nb, b)] = cl
        self.ei_tot = max(icol, 16)


def _mk_M_wb(counts8):
    """counts8: [NCORES, NWIN*NBANK] -> per-(window,bank) tile counts."""
    mx = counts8.max(axis=0).reshape(NWIN, NBANK)
    return tuple(
        tuple(int(-(-mx[w, b] // P)) for b in range(NBANK)) for w in range(NWIN)
    )


# ---------------------------------------------------------------- host side


def _graph_prep(edge_index):
    ei = np.asarray(edge_index).astype(np.int64)
    row, col = ei[0], ei[1]
    loops = np.arange(N_NODES, dtype=np.int64)
    deg = np.bincount(np.concatenate([row, loops]), minlength=N_NODES)
    dis = 1.0 / np.sqrt(np.maximum(deg, 1.0))
    zt_all = np.bincount(col, weights=dis[row], minlength=N_NODES) + dis
    zt_all = zt_all.astype(np.float32)
    dis = dis.astype(np.float32)

    core = col // NPC
    per_core = []
    counts8 = np.zeros((NCORES, NWIN * NBANK), np.int64)
    for c in range(NCORES):
        msk = core == c
        r_c = row[msk]
        d_c = col[msk] - c * NPC
        key = ((d_c // P) * NBANK + r_c // BANK).astype(np.int64)
        order = np.argsort(key, kind="stable")
        r_c, d_c, key = r_c[order], d_c[order], key[order]
        cnt = np.bincount(key, minlength=NWIN * NBANK)
        counts8[c] = cnt
        per_core.append((r_c, d_c, key, cnt))
    M_wb = _mk_M_wb(counts8)
    L = Layout(M_wb)

    # per-(nb, b, w_) base tables for vectorized scatter of edge metadata
    blk_base = np.zeros((NB, NBANK), np.int64)
    wb_base = np.zeros((NB, NBANK, BW), np.int64)
    callbase = np.zeros((NB, NBANK, BW), np.int64)   # i16 col base of call
    tile_off = np.zeros((NB, NBANK, BW), np.int64)   # tiles before window in call
    for nb in range(NB):
        for b in range(NBANK):
            blk_base[nb, b] = L.ef_blk_base[(nb, b)]
            for w_ in range(BW):
                wb_base[nb, b, w_] = L.ef_base[(nb, b, w_)]
            for ci, (w0, nwin, nidx) in enumerate(L.calls[(nb, b)]):
                t = 0
                for k in range(nwin):
                    callbase[nb, b, w0 + k] = L.ei_base[(nb, b, ci)]
                    tile_off[nb, b, w0 + k] = t
                    t += M_wb[nb * BW + w0 + k][b]

    eis, efs = [], []
    for c in range(NCORES):
        r_c, d_c, key, cnt = per_core[c]
        starts = np.zeros(NWIN * NBANK, np.int64)
        starts[1:] = np.cumsum(cnt)[:-1]
        slot = np.arange(len(key)) - starts[key]
        win = key // NBANK
        bank = key % NBANK
        nb_e = win // BW
        w_e = win % BW

        ef_l = np.full((P, L.ef_tot), PAD_OFF, np.float32)
        tcol = blk_base[nb_e, bank] + wb_base[nb_e, bank, w_e] + slot // P
        ef_l[slot % P, tcol] = (d_c % P).astype(np.float32)

        ei_l = np.zeros((16, L.ei_tot), np.int16)
        i_in_call = tile_off[nb_e, bank, w_e] * P + slot
        icol = callbase[nb_e, bank, w_e] + i_in_call // 16
        irow = i_in_call % 16
        ei_l[irow, icol] = (r_c - bank * BANK).astype(np.int16)
        ei_l = np.ascontiguousarray(np.tile(ei_l, (8, 1)))

        eis.append(ei_l)
        efs.append(np.ascontiguousarray(ef_l.astype(NPBF)))
    return M_wb, eis, efs, zt_all, dis


# -------------------------------------------------------------- device graph

_GRAPH_CACHE = {}


def _build(M_wb):
    if M_wb in _GRAPH_CACHE:
        return _GRAPH_CACHE[M_wb]
    L = Layout(M_wb)
    WIDE = L.max_tpb * P

    nc = bacc.Bacc(num_swdge_queues=NBANK)
    xd_ext = nc.declare_dram_parameter("xd", [N_NODES, D], BF16, isOutput=False)
    xT_ext = nc.declare_dram_parameter("xT", [D, NPC_PAD], BF16, isOutput=False)
    xdT_ext = nc.declare_dram_parameter("xdT", [D, NPC_PAD], BF16, isOutput=False)
    ei_ext = nc.declare_dram_parameter("ei", [128, L.ei_tot], I16, isOutput=False)
    ef_ext = nc.declare_dram_parameter("ef", [P, L.ef_tot], BF16, isOutput=False)
    z_ext = nc.declare_dram_parameter("z", [NB, 1, BW * P], BF16, isOutput=False)
    dis_ext = nc.declare_dram_parameter("disc", [P, NWIN], F32, isOutput=False)
    w_ext = nc.declare_dram_parameter("wmat", [P, 3 * D + 3], BF16, isOutput=False)
    b_ext = nc.declare_dram_parameter("bvec", [1, 3 * D + 3], BF16, isOutput=False)
    c_ext = nc.declare_dram_parameter("iotaw", [P, WIDE], BF16, isOutput=False)
    out_ext = nc.declare_dram_parameter("out", [NPC_PAD, D], F32, isOutput=True)
    stagT = nc.dram_tensor("stagT", [D, NPC_PAD], BF16)

    AL = mybir.AluOpType
    with tile.TileContext(nc) as tc:
        with (
            tc.tile_pool(name="const", bufs=1) as constp,
            tc.tile_pool(name="xg", bufs=4) as xgp,
            tc.tile_pool(name="eq", bufs=2) as eqp,
            tc.tile_pool(name="s1", bufs=4) as s1p,
            tc.tile_pool(name="zb", bufs=2) as zp,
            tc.tile_pool(name="p2in", bufs=3) as p2inp,
            tc.tile_pool(name="gate", bufs=3) as gatep,
            tc.tile_pool(name="comb", bufs=3) as combp,
            tc.tile_pool(name="ps_acc", bufs=3, space="PSUM") as pp_acc,
            tc.tile_pool(name="ps_mm", bufs=1, space="PSUM") as pp_mm,
        ):
            iota_w = constp.tile([P, WIDE], BF16)
            nc.sync.dma_start(out=iota_w[:], in_=c_ext[:])
            wm = constp.tile([P, 3 * D + 3], BF16)
            nc.sync.dma_start(out=wm[:], in_=w_ext[:])
            WlT = wm[:, 0:D]
            WhT = wm[:, D : 2 * D]
            WiT = wm[:, 2 * D : 3 * D]
            WgT = wm[:, 3 * D : 3 * D + 3]
            bv = constp.tile([1, 3 * D + 3], BF16)
            nc.sync.dma_start(out=bv[:], in_=b_ext[:])
            b_low = bv[:, 0:D]
            b_high = bv[:, D : 2 * D]
            b_id = bv[:, 2 * D : 3 * D]
            b_gate = bv[:, 3 * D : 3 * D + 3]
            ones = constp.tile([1, P], BF16)
            nc.vector.memset(ones[:], 1.0)
            dis_sb = constp.tile([P, NWIN], F32)
            nc.sync.dma_start(out=dis_sb[:], in_=dis_ext[:])
            ei_sb = constp.tile([128, L.ei_tot], I16)
            nc.sync.dma_start(out=ei_sb[:], in_=ei_ext[:])
            ef_sb = constp.tile([P, L.ef_tot], BF16)
            nc.sync.dma_start(out=ef_sb[:], in_=ef_ext[:])

            # ---- phase 1
            for nb in range(NB):
                eq = {}
                for b in range(NBANK):
                    tpb = L.tpb[(nb, b)]
                    if tpb == 0:
                        continue
                    e0 = L.ef_blk_base[(nb, b)]
                    eqt = eqp.tile([P, L.max_tpb, P], BF16, tag=f"eq{b}")
                    nc.vector.tensor_tensor(
                        out=eqt[:, :tpb, :],
                        in0=iota_w[:, : tpb * P].rearrange(
                            "p (t j) -> p t j", j=P
                        ),
                        in1=ef_sb[:, e0 : e0 + tpb].to_broadcast([P, tpb, P]),
                        op=AL.is_equal,
                    )
                    eq[b] = eqt
                xg_cur = {}
                xg_toff = {}
                for w_ in range(BW):
                    w = nb * BW + w_
                    # issue gathers for calls starting at this window
                    for b in range(NBANK):
                        for ci, (w0, nwin, nidx) in enumerate(L.calls[(nb, b)]):
                            if w0 != w_:
                                continue
                            xgt = xgp.tile(
                                [P, MAX_CALL // P, P], BF16, tag=f"xg{b}"
                            )
                            i0 = L.ei_base[(nb, b, ci)]
                            nc.gpsimd.dma_gather(
                                out_ap=xgt[:, : nidx // P, :],
                                in_ap=xd_ext[
                                    b * BANK : min((b + 1) * BANK, N_NODES), :
                                ],
                                idxs_ap=ei_sb[:, i0 : i0 + nidx // 16],
                                num_idxs=nidx,
                                num_idxs_reg=nidx,
                                elem_size=P,
                                queue_num=b,
                            )
                            xg_cur[b] = xgt
                            xg_toff[b] = {}
                            t = 0
                            for k in range(nwin):
                                xg_toff[b][w0 + k] = t
                                t += M_wb[nb * BW + w0 + k][b]
                    ntile_w = sum(M_wb[w][b] for b in range(NBANK))
                    if ntile_w == 0:
                        s_sb = s1p.tile([P, P], BF16, tag="s1")
                        nc.vector.memset(s_sb[:], 0.0)
                        nc.sync.dma_start(
                            out=stagT[:, w * P : (w + 1) * P], in_=s_sb[:]
                        )
                        continue
                    ps = pp_acc.tile([P, P], F32, tag="ps")   # [f, j]
                    k = 0
                    for b in range(NBANK):
                        for m in range(M_wb[w][b]):
                            tg = xg_toff[b][w_] + m
                            te = L.ef_base[(nb, b, w_)] + m
                            nc.tensor.matmul(
                                ps[:],
                                lhsT=xg_cur[b][:, tg, :],
                                rhs=eq[b][:, te, :],
                                start=(k == 0),
                                stop=(k == ntile_w - 1),
                            )
                            k += 1
                    s_sb = s1p.tile([P, P], BF16, tag="s1")
                    nc.scalar.copy(s_sb[:], ps[:])
                    nc.sync.dma_start(
                        out=stagT[:, w * P : (w + 1) * P], in_=s_sb[:]
                    )

            # ---- phase 2
            for nb in range(NB):
                z_sb = zp.tile([1, BW * P], BF16, tag="z")
                nc.scalar.dma_start(out=z_sb[:], in_=z_ext[nb])
                for w_ in range(BW):
                    t = nb * BW + w_
                    c0 = t * P
                    xT_sb = p2inp.tile([P, P], BF16, tag="xT")
                    nc.scalar.dma_start(out=xT_sb[:], in_=xT_ext[:, c0 : c0 + P])
                    st_sb = p2inp.tile([P, P], BF16, tag="st")
                    nc.scalar.dma_start(out=st_sb[:], in_=stagT[:, c0 : c0 + P])
                    xdT_sb = p2inp.tile([P, P], BF16, tag="xdT")
                    nc.scalar.dma_start(out=xdT_sb[:], in_=xdT_ext[:, c0 : c0 + P])
                    sT2 = p2inp.tile([P, P], BF16, tag="sT2")
                    nc.vector.tensor_tensor(
                        out=sT2[:], in0=st_sb[:], in1=xdT_sb[:], op=AL.add
                    )
                    zrow = z_sb[0:1, w_ * P : (w_ + 1) * P]

                    ps_low = pp_mm.tile([P, P], F32, tag="ps_low")
                    nc.tensor.matmul(ps_low[:], lhsT=sT2[:], rhs=WlT, start=True, stop=False)
                    nc.tensor.matmul(ps_low[:], lhsT=zrow, rhs=b_low, start=False, stop=True)
                    ps_hl = pp_mm.tile([P, P], F32, tag="ps_hl")
                    nc.tensor.matmul(ps_hl[:], lhsT=sT2[:], rhs=WhT, start=True, stop=False)
                    nc.tensor.matmul(ps_hl[:], lhsT=zrow, rhs=b_high, start=False, stop=True)
                    ps_high = pp_mm.tile([P, P], F32, tag="ps_high")
                    nc.tensor.matmul(ps_high[:], lhsT=xT_sb[:], rhs=WhT, start=True, stop=False)
                    nc.tensor.matmul(ps_high[:], lhsT=ones[:], rhs=b_high, start=False, stop=True)
                    ps_id = pp_mm.tile([P, P], F32, tag="ps_id")
                    nc.tensor.matmul(ps_id[:], lhsT=xT_sb[:], rhs=WiT, start=True, stop=False)
                    nc.tensor.matmul(ps_id[:], lhsT=ones[:], rhs=b_id, start=False, stop=True)
                    ps_gate = pp_mm.tile([P, 3], F32, tag="ps_gate")
                    nc.tensor.matmul(ps_gate[:], lhsT=xT_sb[:], rhs=WgT, start=True, stop=False)
                    nc.tensor.matmul(ps_gate[:], lhsT=ones[:], rhs=b_gate, start=False, stop=True)

                    eg = gatep.tile([P, 3], F32, tag="eg")
                    nc.scalar.activation(
                        eg[:], ps_gate[:], mybir.ActivationFunctionType.Exp
                    )
                    gs = gatep.tile([P, 1], F32, tag="gs")
                    nc.vector.tensor_reduce(
                        out=gs[:], in_=eg[:], axis=mybir.AxisListType.X, op=AL.add
                    )
                    gr = gatep.tile([P, 1], F32, tag="gr")
                    nc.vector.reciprocal(gr[:], gs[:])
                    g = gatep.tile([P, 3], F32, tag="g")
                    nc.vector.tensor_scalar(
                        out=g[:], in0=eg[:], scalar1=gr[:, 0:1], scalar2=None,
                        op0=AL.mult,
                    )
                    gdis = gatep.tile([P, 2], F32, tag="gdis")
                    nc.vector.tensor_scalar(
                        out=gdis[:], in0=g[:, 0:2], scalar1=dis_sb[:, t : t + 1],
                        scalar2=None, op0=AL.mult,
                    )

                    u = combp.tile([P, P], F32, tag="u")
                    nc.scalar.activation(
                        u[:], ps_low[:], mybir.ActivationFunctionType.Copy,
                        scale=gdis[:, 0:1],
                    )
                    v1 = combp.tile([P, P], F32, tag="v1")
                    nc.scalar.activation(
                        v1[:], ps_high[:], mybir.ActivationFunctionType.Copy,
                        scale=g[:, 1:2],
                    )
                    v2 = combp.tile([P, P], F32, tag="v2")
                    nc.vector.tensor_scalar(
                        out=v2[:], in0=ps_hl[:], scalar1=gdis[:, 1:2], scalar2=None,
                        op0=AL.mult,
                    )
                    w2 = combp.tile([P, P], F32, tag="w2")
                    nc.scalar.activation(
                        w2[:], ps_id[:], mybir.ActivationFunctionType.Copy,
                        scale=g[:, 2:3],
                    )
                    o = combp.tile([P, P], F32, tag="o")
                    nc.vector.tensor_tensor(out=o[:], in0=u[:], in1=v1[:], op=AL.add)
                    nc.vector.tensor_tensor(out=o[:], in0=o[:], in1=v2[:], op=AL.subtract)
                    nc.vector.tensor_tensor(out=o[:], in0=o[:], in1=w2[:], op=AL.add)
                    nc.sync.dma_start(out=out_ext[c0 : c0 + P, :], in_=o[:])

    nc.compile()
    _GRAPH_CACHE[M_wb] = nc
    return nc


# -------------------------------------------------------------------- entry


def _reset_device():
    try:
        import ctypes
        import jax

        lib = ctypes.CDLL("/opt/axon/libaxon_pjrt.so")
        if hasattr(lib, "axon_reset"):
            jax.devices()
            lib.axon_reset.restype = ctypes.c_int64
            lib.axon_reset()
    except Exception:
        pass


def kernel(x, edge_index, W_low, b_low, W_high, b_high, W_id, b_id, W_gate, b_gate):
    x = np.asarray(x, dtype=np.float32)
    M_wb, eis, efs, zt_all, dis = _graph_prep(edge_index)
    nc = _build(M_wb)
    L = Layout(M_wb)

    xd = np.ascontiguousarray((dis[:, None] * x).astype(NPBF))
    wmat = np.ascontiguousarray(
        np.concatenate(
            [
                np.asarray(W_low, np.float32).T,
                np.asarray(W_high, np.float32).T,
                np.asarray(W_id, np.float32).T,
                np.asarray(W_gate, np.float32).T,
            ],
            axis=1,
        ).astype(NPBF)
    )
    bvec = np.ascontiguousarray(
        np.concatenate(
            [
                np.asarray(b_low, np.float32),
                np.asarray(b_high, np.float32),
                np.asarray(b_id, np.float32),
                np.asarray(b_gate, np.float32),
            ]
        )[None, :].astype(NPBF)
    )
    iotaw = np.ascontiguousarray(
        np.tile(np.arange(P, dtype=np.float32), (P, L.max_tpb)).astype(NPBF)
    )

    in_maps = []
    for c in range(NCORES):
        lo = c * NPC
        xp = np.zeros((NPC_PAD, D), np.float32)
        xp[:NPC] = x[lo : lo + NPC]
        xT = np.ascontiguousarray(xp.T.astype(NPBF))
        xdp = np.zeros((NPC_PAD, D), np.float32)
        xdp[:NPC] = dis[lo : lo + NPC, None] * x[lo : lo + NPC]
        xdT = np.ascontiguousarray(xdp.T.astype(NPBF))
        zt = np.zeros(NPC_PAD, np.float32)
        zt[:NPC] = zt_all[lo : lo + NPC]
        zt = np.ascontiguousarray(zt.reshape(NB, 1, BW * P).astype(NPBF))
        dc = np.zeros(NPC_PAD, np.float32)
        dc[:NPC] = dis[lo : lo + NPC]
        disc = np.ascontiguousarray(dc.reshape(NWIN, P).T)
        in_maps.append(
            dict(
                xd=xd,
                xT=xT,
                xdT=xdT,
                ei=eis[c],
                ef=efs[c],
                z=zt,
                disc=disc,
                wmat=wmat,
                bvec=bvec,
                iotaw=iotaw,
            )
        )

    try:
        res = run_bass_kernel_spmd(nc, in_maps, list(range(NCORES)))
    except Exception:
        # a previous crashed run may have wedged the device; reset and retry
        _reset_device()
        res = run_bass_kernel_spmd(nc, in_maps, list(range(NCORES)))
    out = np.concatenate(
        [res.results[c]["out"][:NPC] for c in range(NCORES)], axis=0
    )
    return out


# revision 14
# speedup vs baseline: 3.1361x; 1.2502x over previous
"""ACMConv (adaptive channel mixing GCN layer) on 8 Trainium2 NeuronCores.

Strategy (graph/data parallel, edges partitioned by destination):
- Host: compute GCN norms with self loops; factorize
  norm_e = dis[src] * dis[dst]. The src factor is folded into the bf16
  gather table xd = dis[:,None]*x; the dst factor is applied in phase 2.
  Non-loop edges are partitioned by destination core (node blocks of
  12500), grouped into 128-destination windows, and split by source bank
  (int16 dma_gather indices address 32768-row banks). Each (window, bank)
  segment is padded to M_wb 128-edge tiles, where M_wb is the max over
  the 8 cores for that (window, bank) -> identical SPMD graph, minimal
  padding. Segments of consecutive windows are packed into dma_gather
  calls of <= 1024 indices, issued on SWDGE queue b (4 Q7 pairs work in
  parallel).
- Device phase 1: per call one dma_gather pulls the source rows (idx 0
  for padding); per (block, bank) one wide tensor_tensor builds the
  one-hot eq[e, j] = (dest_off_e == j) (999 offsets kill padding); the
  tensor engine accumulates psum[f, j] += xg.T @ eq over the window's
  tiles, yielding the aggregation s~ pre-transposed; an ACT copy casts it
  to bf16 staging sT~[128, nodes].
- Device phase 2: per 128-node tile, s~ += dis*x (self loops, host-shipped
  xdT), then bf16 matmuls s~@W_low.T, s~@W_high.T, x@W_high.T, x@W_id.T,
  x@W_gate.T (+ bias rank-1 terms via K=1 matmuls, z~-weighted for
  aggregated terms), softmax gate, and combine with the dst factor dis
  folded into the gate coefficients of the aggregated terms.
Output rows are node-contiguous so the host just concatenates core shards.
"""

import numpy as np
import ml_dtypes

import concourse.bass as bass
import concourse.bacc as bacc
import concourse.mybir as mybir
import concourse.tile as tile
from concourse.bass_utils import run_bass_kernel_spmd

N_NODES = 100000
D = 128
NCORES = 8
NPC = N_NODES // NCORES            # 12500 nodes per core
P = 128
NWIN = (NPC + P - 1) // P          # 98 destination windows per core
NPC_PAD = NWIN * P                 # 12544
BW = 14                            # windows per block
NB = NWIN // BW                    # 7 blocks
NBANK = 4
BANK = 25000                      # equal banks -> balanced (window, bank) cells
MAX_CALL = 1024                    # dma_gather Q7 scratch limit
PAD_OFF = 999.0                    # kills one-hot for padding slots
assert NB * BW == NWIN

F32 = mybir.dt.float32
BF16 = mybir.dt.bfloat16
I16 = mybir.dt.int16
NPBF = ml_dtypes.bfloat16


class Layout:
    """Derived, deterministic layout from the (window, bank) tile map."""

    def __init__(self, M_wb):
        self.M_wb = M_wb                       # [NWIN][NBANK] ints
        # ef tile columns: per (block, bank) ragged run of tiles
        self.ef_base = {}                      # (nb, b, w_) -> tile col base
        self.tpb = {}                          # (nb, b) -> tiles in block/bank
        self.ef_blk_base = {}                  # (nb, b) -> global ef col base
        col = 0
        self.max_tpb = 0
        for nb in range(NB):
            for b in range(NBANK):
                self.ef_blk_base[(nb, b)] = col
                t = 0
                for w_ in range(BW):
                    self.ef_base[(nb, b, w_)] = t
                    t += M_wb[nb * BW + w_][b]
                self.tpb[(nb, b)] = t
                self.max_tpb = max(self.max_tpb, t)
                col += t
        self.ef_tot = col
        # gather calls: per (block, bank) greedy pack windows, <=1024 idxs
        self.calls = {}                        # (nb, b) -> list of (w0, nwin, nidx)
        self.ei_base = {}                      # (nb, b, call_i) -> i16 col base
        icol = 0
        for nb in range(NB):
            for b in range(NBANK):
                cl = []
                w_ = 0
                while w_ < BW:
                    n = 0
                    nidx = 0
                    while w_ + n < BW:
                        cap = M_wb[nb * BW + w_ + n][b] * P
                        if n > 0 and nidx + cap > MAX_CALL:
                            break
                        assert cap <= MAX_CALL
                        nidx += cap
                        n += 1
                    if nidx > 0:
                        self.ei_base[(nb, b, len(cl))] = icol
                        icol += nidx // 16
                        cl.append((w_, n, nidx))
                    w_ += max(n, 1)
                self.calls[(nb, b)] = cl
        self.ei_tot = max(icol, 16)


def _mk_M_wb(counts8):
    """counts8: [NCORES, NWIN*NBANK] -> per-(window,bank) tile counts."""
    mx = counts8.max(axis=0).reshape(NWIN, NBANK)
    return tuple(
        tuple(int(-(-mx[w, b] // P)) for b in range(NBANK)) for w in range(NWIN)
    )


# ---------------------------------------------------------------- host side


def _graph_prep(edge_index):
    ei = np.asarray(edge_index).astype(np.int64)
    row, col = ei[0], ei[1]
    loops = np.arange(N_NODES, dtype=np.int64)
    deg = np.bincount(np.concatenate([row, loops]), minlength=N_NODES)
    dis = 1.0 / np.sqrt(np.maximum(deg, 1.0))
    zt_all = np.bincount(col, weights=dis[row], minlength=N_NODES) + dis
    zt_all = zt_all.astype(np.float32)
    dis = dis.astype(np.float32)

    core = col // NPC
    per_core = []
    counts8 = np.zeros((NCORES, NWIN * NBANK), np.int64)
    for c in range(NCORES):
        msk = core == c
        r_c = row[msk]
        d_c = col[msk] - c * NPC
        key = ((d_c // P) * NBANK + r_c // BANK).astype(np.int64)
        order = np.argsort(key, kind="stable")
        r_c, d_c, key = r_c[order], d_c[order], key[order]
        cnt = np.bincount(key, minlength=NWIN * NBANK)
        counts8[c] = cnt
        per_core.append((r_c, d_c, key, cnt))
    M_wb = _mk_M_wb(counts8)
    L = Layout(M_wb)

    # per-(nb, b, w_) base tables for vectorized scatter of edge metadata
    blk_base = np.zeros((NB, NBANK), np.int64)
    wb_base = np.zeros((NB, NBANK, BW), np.int64)
    callbase = np.zeros((NB, NBANK, BW), np.int64)   # i16 col base of call
    tile_off = np.zeros((NB, NBANK, BW), np.int64)   # tiles before window in call
    for nb in range(NB):
        for b in range(NBANK):
            blk_base[nb, b] = L.ef_blk_base[(nb, b)]
            for w_ in range(BW):
                wb_base[nb, b, w_] = L.ef_base[(nb, b, w_)]
            for ci, (w0, nwin, nidx) in enumerate(L.calls[(nb, b)]):
                t = 0
                for k in range(nwin):
                    callbase[nb, b, w0 + k] = L.ei_base[(nb, b, ci)]
                    tile_off[nb, b, w0 + k] = t
                    t += M_wb[nb * BW + w0 + k][b]

    eis, efs = [], []
    for c in range(NCORES):
        r_c, d_c, key, cnt = per_core[c]
        starts = np.zeros(NWIN * NBANK, np.int64)
        starts[1:] = np.cumsum(cnt)[:-1]
        slot = np.arange(len(key)) - starts[key]
        win = key // NBANK
        bank = key % NBANK
        nb_e = win // BW
        w_e = win % BW

        ef_l = np.full((P, L.ef_tot), PAD_OFF, np.float32)
        tcol = blk_base[nb_e, bank] + wb_base[nb_e, bank, w_e] + slot // P
        ef_l[slot % P, tcol] = (d_c % P).astype(np.float32)

        ei_l = np.zeros((16, L.ei_tot), np.int16)
        i_in_call = tile_off[nb_e, bank, w_e] * P + slot
        icol = callbase[nb_e, bank, w_e] + i_in_call // 16
        irow = i_in_call % 16
        ei_l[irow, icol] = (r_c - bank * BANK).astype(np.int16)
        ei_l = np.ascontiguousarray(np.tile(ei_l, (8, 1)))

        eis.append(ei_l)
        efs.append(np.ascontiguousarray(ef_l.astype(NPBF)))
    return M_wb, eis, efs, zt_all, dis


# -------------------------------------------------------------- device graph

_GRAPH_CACHE = {}


def _build(M_wb):
    if M_wb in _GRAPH_CACHE:
        return _GRAPH_CACHE[M_wb]
    L = Layout(M_wb)
    WIDE = L.max_tpb * P

    nc = bacc.Bacc(num_swdge_queues=NBANK)
    xd_ext = nc.declare_dram_parameter("xd", [N_NODES, D], BF16, isOutput=False)
    xx_ext = nc.declare_dram_parameter("xx", [D, 2 * NPC_PAD], BF16, isOutput=False)
    ei_ext = nc.declare_dram_parameter("ei", [128, L.ei_tot], I16, isOutput=False)
    ef_ext = nc.declare_dram_parameter("ef", [P, L.ef_tot], BF16, isOutput=False)
    z_ext = nc.declare_dram_parameter("z", [NB, 1, BW * P], BF16, isOutput=False)
    dis_ext = nc.declare_dram_parameter("disc", [P, NWIN], F32, isOutput=False)
    w_ext = nc.declare_dram_parameter("wmat", [P, 3 * D + 3], BF16, isOutput=False)
    b_ext = nc.declare_dram_parameter("bvec", [1, 3 * D + 3], BF16, isOutput=False)
    c_ext = nc.declare_dram_parameter("iotaw", [P, WIDE], BF16, isOutput=False)
    out_ext = nc.declare_dram_parameter("out", [NPC_PAD, D], BF16, isOutput=True)

    AL = mybir.AluOpType
    with tile.TileContext(nc) as tc:
        with (
            tc.tile_pool(name="const", bufs=1) as constp,
            tc.tile_pool(name="xg", bufs=4) as xgp,
            tc.tile_pool(name="eq", bufs=2) as eqp,
            tc.tile_pool(name="zb", bufs=2) as zp,
            tc.tile_pool(name="p2in", bufs=4) as p2inp,
            tc.tile_pool(name="gate", bufs=3) as gatep,
            tc.tile_pool(name="comb", bufs=3) as combp,
            tc.tile_pool(name="ps_acc", bufs=3, space="PSUM") as pp_acc,
            tc.tile_pool(name="ps_mm", bufs=2, space="PSUM") as pp_mm,
        ):
            iota_w = constp.tile([P, WIDE], BF16)
            nc.sync.dma_start(out=iota_w[:], in_=c_ext[:])
            wm = constp.tile([P, 3 * D + 3], BF16)
            nc.sync.dma_start(out=wm[:], in_=w_ext[:])
            WlT = wm[:, 0:D]
            WhT = wm[:, D : 2 * D]
            WiT = wm[:, 2 * D : 3 * D]
            WgT = wm[:, 3 * D : 3 * D + 3]
            bv = constp.tile([1, 3 * D + 3], BF16)
            nc.sync.dma_start(out=bv[:], in_=b_ext[:])
            b_low = bv[:, 0:D]
            b_high = bv[:, D : 2 * D]
            b_id = bv[:, 2 * D : 3 * D]
            b_gate = bv[:, 3 * D : 3 * D + 3]
            ones = constp.tile([1, P], BF16)
            nc.vector.memset(ones[:], 1.0)
            dis_sb = constp.tile([P, NWIN], F32)
            nc.sync.dma_start(out=dis_sb[:], in_=dis_ext[:])
            ei_sb = constp.tile([128, L.ei_tot], I16)
            nc.sync.dma_start(out=ei_sb[:], in_=ei_ext[:])
            ef_sb = constp.tile([P, L.ef_tot], BF16)
            nc.sync.dma_start(out=ef_sb[:], in_=ef_ext[:])

            # ---- fused per-window: aggregate + project + gate + combine
            for nb in range(NB):
                eq = {}
                for b in range(NBANK):
                    tpb = L.tpb[(nb, b)]
                    if tpb == 0:
                        continue
                    e0 = L.ef_blk_base[(nb, b)]
                    eqt = eqp.tile([P, L.max_tpb, P], BF16, tag=f"eq{b}")
                    nc.vector.tensor_tensor(
                        out=eqt[:, :tpb, :],
                        in0=iota_w[:, : tpb * P].rearrange(
                            "p (t j) -> p t j", j=P
                        ),
                        in1=ef_sb[:, e0 : e0 + tpb].to_broadcast([P, tpb, P]),
                        op=AL.is_equal,
                    )
                    eq[b] = eqt
                z_sb = zp.tile([1, BW * P], BF16, tag="z")
                nc.scalar.dma_start(out=z_sb[:], in_=z_ext[nb])
                xg_cur = {}
                xg_toff = {}
                o_pair = None
                for w_ in range(BW):
                    w = nb * BW + w_
                    # issue gathers for calls starting at this window
                    for b in range(NBANK):
                        for ci, (w0, nwin, nidx) in enumerate(L.calls[(nb, b)]):
                            if w0 != w_:
                                continue
                            xgt = xgp.tile(
                                [P, MAX_CALL // P, P], BF16, tag=f"xg{b}"
                            )
                            i0 = L.ei_base[(nb, b, ci)]
                            nc.gpsimd.dma_gather(
                                out_ap=xgt[:, : nidx // P, :],
                                in_ap=xd_ext[
                                    b * BANK : min((b + 1) * BANK, N_NODES), :
                                ],
                                idxs_ap=ei_sb[:, i0 : i0 + nidx // 16],
                                num_idxs=nidx,
                                num_idxs_reg=nidx,
                                elem_size=P,
                                queue_num=b,
                            )
                            xg_cur[b] = xgt
                            xg_toff[b] = {}
                            t = 0
                            for k in range(nwin):
                                xg_toff[b][w0 + k] = t
                                t += M_wb[nb * BW + w0 + k][b]
                    # interleaved x.T | (dis*x).T for this window
                    xx_sb = p2inp.tile([P, 2 * P], BF16, tag="xx")
                    nc.sync.dma_start(
                        out=xx_sb[:], in_=xx_ext[:, 2 * w * P : 2 * (w + 1) * P]
                    )
                    xT_sl = xx_sb[:, 0:P]
                    xdT_sl = xx_sb[:, P : 2 * P]

                    ntile_w = sum(M_wb[w][b] for b in range(NBANK))
                    sT2 = p2inp.tile([P, P], BF16, tag="sT2")
                    if ntile_w == 0:
                        nc.vector.tensor_copy(out=sT2[:], in_=xdT_sl)
                    else:
                        ps = pp_acc.tile([P, P], F32, tag="ps")   # [f, j]
                        k = 0
                        for b in range(NBANK):
                            for m in range(M_wb[w][b]):
                                tg = xg_toff[b][w_] + m
                                te = L.ef_base[(nb, b, w_)] + m
                                nc.tensor.matmul(
                                    ps[:],
                                    lhsT=xg_cur[b][:, tg, :],
                                    rhs=eq[b][:, te, :],
                                    start=(k == 0),
                                    stop=(k == ntile_w - 1),
                                )
                                k += 1
                        # s~ + dis*x (self loop), psum f32 + sbuf bf16 -> bf16
                        nc.vector.tensor_tensor(
                            out=sT2[:], in0=ps[:], in1=xdT_sl, op=AL.add
                        )
                    zrow = z_sb[0:1, w_ * P : (w_ + 1) * P]

                    # projections: one wide psum, 4 x 128-col ranges
                    pm = pp_mm.tile([P, 4 * P], F32, tag="pm")
                    ps_low = pm[:, 0:P]
                    ps_hl = pm[:, P : 2 * P]
                    ps_high = pm[:, 2 * P : 3 * P]
                    ps_id = pm[:, 3 * P : 4 * P]
                    nc.tensor.matmul(ps_low, lhsT=sT2[:], rhs=WlT, start=True, stop=False, skip_group_check=True)
                    nc.tensor.matmul(ps_low, lhsT=zrow, rhs=b_low, start=False, stop=True, skip_group_check=True)
                    nc.tensor.matmul(ps_hl, lhsT=sT2[:], rhs=WhT, start=True, stop=False, skip_group_check=True)
                    nc.tensor.matmul(ps_hl, lhsT=zrow, rhs=b_high, start=False, stop=True, skip_group_check=True)
                    nc.tensor.matmul(ps_high, lhsT=xT_sl, rhs=WhT, start=True, stop=False, skip_group_check=True)
                    nc.tensor.matmul(ps_high, lhsT=ones[:], rhs=b_high, start=False, stop=True, skip_group_check=True)
                    nc.tensor.matmul(ps_id, lhsT=xT_sl, rhs=WiT, start=True, stop=False, skip_group_check=True)
                    nc.tensor.matmul(ps_id, lhsT=ones[:], rhs=b_id, start=False, stop=True, skip_group_check=True)
                    ps_gate = pp_mm.tile([P, 3], F32, tag="ps_gate")
                    nc.tensor.matmul(ps_gate[:], lhsT=xT_sl, rhs=WgT, start=True, stop=False)
                    nc.tensor.matmul(ps_gate[:], lhsT=ones[:], rhs=b_gate, start=False, stop=True)

                    eg = gatep.tile([P, 3], F32, tag="eg")
                    nc.scalar.activation(
                        eg[:], ps_gate[:], mybir.ActivationFunctionType.Exp
                    )
                    gs = gatep.tile([P, 1], F32, tag="gs")
                    nc.vector.tensor_reduce(
                        out=gs[:], in_=eg[:], axis=mybir.AxisListType.X, op=AL.add
                    )
                    gr = gatep.tile([P, 1], F32, tag="gr")
                    nc.vector.reciprocal(gr[:], gs[:])
                    g = gatep.tile([P, 3], F32, tag="g")
                    nc.vector.tensor_scalar(
                        out=g[:], in0=eg[:], scalar1=gr[:, 0:1], scalar2=None,
                        op0=AL.mult,
                    )
                    gdis = gatep.tile([P, 2], F32, tag="gdis")
                    nc.vector.tensor_scalar(
                        out=gdis[:], in0=g[:, 0:2], scalar1=dis_sb[:, w : w + 1],
                        scalar2=None, op0=AL.mult,
                    )

                    u = combp.tile([P, P], BF16, tag="u")
                    nc.scalar.activation(
                        u[:], ps_low, mybir.ActivationFunctionType.Copy,
                        scale=gdis[:, 0:1],
                    )
                    v1 = combp.tile([P, P], BF16, tag="v1")
                    nc.scalar.activation(
                        v1[:], ps_high, mybir.ActivationFunctionType.Copy,
                        scale=g[:, 1:2],
                    )
                    v2 = combp.tile([P, P], BF16, tag="v2")
                    nc.vector.tensor_scalar(
                        out=v2[:], in0=ps_hl, scalar1=gdis[:, 1:2], scalar2=None,
                        op0=AL.mult,
                    )
                    w2 = combp.tile([P, P], BF16, tag="w2")
                    nc.scalar.activation(
                        w2[:], ps_id, mybir.ActivationFunctionType.Copy,
                        scale=g[:, 2:3],
                    )
                    if w_ % 2 == 0:
                        o_pair = combp.tile([P, 2 * P], BF16, tag="o")
                    osl = o_pair[:, (w_ % 2) * P : (w_ % 2 + 1) * P]
                    nc.vector.tensor_tensor(out=osl, in0=u[:], in1=v1[:], op=AL.add)
                    nc.vector.tensor_tensor(out=osl, in0=osl, in1=v2[:], op=AL.subtract)
                    nc.vector.tensor_tensor(out=osl, in0=osl, in1=w2[:], op=AL.add)
                    if w_ % 2 == 1:
                        c0 = (w - 1) * P
                        nc.sync.dma_start(
                            out=out_ext[c0 : c0 + 2 * P, :].rearrange(
                                "(k p) f -> p k f", k=2
                            ),
                            in_=o_pair[:].rearrange("p (k f) -> p k f", k=2),
                        )

    nc.compile()
    _GRAPH_CACHE[M_wb] = nc
    return nc


# -------------------------------------------------------------------- entry


def _reset_device():
    try:
        import ctypes
        import jax

        lib = ctypes.CDLL("/opt/axon/libaxon_pjrt.so")
        if hasattr(lib, "axon_reset"):
            jax.devices()
            lib.axon_reset.restype = ctypes.c_int64
            lib.axon_reset()
    except Exception:
        pass


def kernel(x, edge_index, W_low, b_low, W_high, b_high, W_id, b_id, W_gate, b_gate):
    x = np.asarray(x, dtype=np.float32)
    M_wb, eis, efs, zt_all, dis = _graph_prep(edge_index)
    nc = _build(M_wb)
    L = Layout(M_wb)

    xd = np.ascontiguousarray((dis[:, None] * x).astype(NPBF))
    wmat = np.ascontiguousarray(
        np.concatenate(
            [
                np.asarray(W_low, np.float32).T,
                np.asarray(W_high, np.float32).T,
                np.asarray(W_id, np.float32).T,
                np.asarray(W_gate, np.float32).T,
            ],
            axis=1,
        ).astype(NPBF)
    )
    bvec = np.ascontiguousarray(
        np.concatenate(
            [
                np.asarray(b_low, np.float32),
                np.asarray(b_high, np.float32),
                np.asarray(b_id, np.float32),
                np.asarray(b_gate, np.float32),
            ]
        )[None, :].astype(NPBF)
    )
    iotaw = np.ascontiguousarray(
        np.tile(np.arange(P, dtype=np.float32), (P, L.max_tpb)).astype(NPBF)
    )

    in_maps = []
    for c in range(NCORES):
        lo = c * NPC
        xp = np.zeros((NPC_PAD, D), np.float32)
        xp[:NPC] = x[lo : lo + NPC]
        xdp = np.zeros((NPC_PAD, D), np.float32)
        xdp[:NPC] = dis[lo : lo + NPC, None] * x[lo : lo + NPC]
        # interleave per window: [.., xT_w (128 cols), xdT_w (128 cols), ..]
        xx = np.empty((D, 2 * NPC_PAD), np.float32)
        xTv = xp.T.reshape(D, NWIN, P)
        xdTv = xdp.T.reshape(D, NWIN, P)
        xx = np.stack([xTv, xdTv], axis=2).reshape(D, 2 * NPC_PAD)
        xx = np.ascontiguousarray(xx.astype(NPBF))
        zt = np.zeros(NPC_PAD, np.float32)
        zt[:NPC] = zt_all[lo : lo + NPC]
        zt = np.ascontiguousarray(zt.reshape(NB, 1, BW * P).astype(NPBF))
        dc = np.zeros(NPC_PAD, np.float32)
        dc[:NPC] = dis[lo : lo + NPC]
        disc = np.ascontiguousarray(dc.reshape(NWIN, P).T)
        in_maps.append(
            dict(
                xd=xd,
                xx=xx,
                ei=eis[c],
                ef=efs[c],
                z=zt,
                disc=disc,
                wmat=wmat,
                bvec=bvec,
                iotaw=iotaw,
            )
        )

    try:
        res = run_bass_kernel_spmd(nc, in_maps, list(range(NCORES)))
    except Exception:
        # a previous crashed run may have wedged the device; reset and retry
        _reset_device()
        res = run_bass_kernel_spmd(nc, in_maps, list(range(NCORES)))
    out = np.concatenate(
        [res.results[c]["out"][:NPC] for c in range(NCORES)], axis=0
    )
    return out.astype(np.float32)
